# revision 1
# baseline (speedup 1.0000x reference)
"""Trainium2 Bass kernel for CGRE-style ragged bag attention pooling + classifier.

Computation (per reference):
    seg[i]   : bag of sentence i (contiguous ragged scopes)
    s[i]     = X[i] . Constraints[X_Rel[seg[i]]]
    w[i]     = softmax of s within bag (numerically stabilized per bag)
    bag[b]   = sum_{i in b} w[i] * X[i]
    out      = bag @ W.T + b

Strategy: 8-way data parallel over contiguous bag ranges (1024 bags/core).
One shared SPMD Bass program; all per-core raggedness (scope boundaries,
bag windows, gather offsets) is passed as per-core *data* (index tensors),
never baked into the program.

Per core, sentences are processed in 128-row tiles, 4 tiles per "group".
Each group owns a 128-bag window starting at the bag of its first sentence
(windows of adjacent groups overlap; straddling bags get partial sums from
both groups which are combined by a CCE-add indirect scatter into DRAM).

Key identity used for the classifier: out[b] = rw_b * sum_i e_i * (X_i . W_r)
with the bag pooled in PSUM [bag-window, C], then PE-transposed and folded
with W.T so no full-size transpose of X is ever needed.
"""

import numpy as np
from contextlib import ExitStack

import ml_dtypes

import concourse.bass as bass
import concourse.tile as tile
from concourse import bacc, mybir
from concourse.bass import IndirectOffsetOnAxis
from concourse.bass_utils import run_bass_kernel_spmd
from concourse.masks import make_identity
from concourse.tile import add_dep_helper

NCORES = 8
P = 128
TPG = 4          # tiles per group (512 sentences)
F32 = mybir.dt.float32
BF16 = mybir.dt.bfloat16
I32 = mybir.dt.int32


# ----------------------------------------------------------------------------
# Host-side preparation
# ----------------------------------------------------------------------------

def _prep(X, Constraints, W, b, X_Scope, X_Rel):
    N, C = X.shape
    R = Constraints.shape[0]
    B = X_Scope.shape[0]
    assert B % NCORES == 0
    NB_LOC = B // NCORES

    starts = np.asarray(X_Scope[:, 0], dtype=np.int64)
    ends = np.asarray(X_Scope[:, 1], dtype=np.int64)
    lens = ends - starts
    seg = np.searchsorted(starts, np.arange(N), side="right") - 1
    rel_s = np.asarray(X_Rel, dtype=np.int64)[seg]          # relation per sentence

    LW = int(max(64, ((lens.max() + 31) // 32) * 32))        # gather row width

    core_b0 = [k * NB_LOC for k in range(NCORES)]
    core_s0 = [int(starts[b0]) for b0 in core_b0]
    core_s1 = [int(ends[b0 + NB_LOC - 1]) for b0 in core_b0]
    cnts = [s1 - s0 for s0, s1 in zip(core_s0, core_s1)]

    # pick group size (sentences) so every group's bags fit a 128-bag window
    GS = TPG * P
    SMAX = ((max(cnts) + GS - 1) // GS) * GS
    T = SMAX // P
    NG = T // TPG

    def win_ok(gs):
        for k in range(NCORES):
            s0, cnt = core_s0[k], cnts[k]
            for g0 in range(0, cnt, gs):
                lo = seg[s0 + g0]
                hi = seg[s0 + min(g0 + gs, cnt) - 1]
                if hi - lo + 1 > P:
                    return False
        return True

    assert win_ok(GS), (
        "128-bag window does not cover a 512-sentence group; "
        "scope distribution far from expected"
    )

    X = np.ascontiguousarray(np.asarray(X, dtype=np.float32))
    cons_np = np.ascontiguousarray(np.asarray(Constraints, dtype=np.float32))
    wt_np = np.ascontiguousarray(np.asarray(W, dtype=np.float32).T)  # [C, R]

    in_maps = []
    for k in range(NCORES):
        s0, s1, cnt = core_s0[k], core_s1[k], cnts[k]
        b0 = core_b0[k]

        xpad = np.zeros((SMAX, C), dtype=np.float32)
        xpad[:cnt] = X[s0:s1]

        jj = np.arange(cnt)
        tt = jj // P
        pp = jj % P
        segl = seg[s0:s1] - b0                               # local bag ids [0, NB_LOC)

        # group windows
        wlo = np.zeros(NG, dtype=np.int64)                   # local window base bag
        whi = np.full(NG, -1, dtype=np.int64)                # local last real bag
        for g in range(NG):
            g0 = g * GS
            if g0 >= cnt:
                wlo[g] = 0
                whi[g] = -1
            else:
                wlo[g] = segl[g0]
                whi[g] = segl[min(g0 + GS, cnt) - 1]

        segloc = np.full((P, T), -1.0, dtype=np.float32)     # [p, t]
        segloc[pp, tt] = (segl - wlo[tt // TPG]).astype(np.float32)

        ohrel = np.zeros((T, R, P), dtype=np.float32)
        ohrel[tt, rel_s[s0:s1], pp] = 1.0

        ohexp = np.zeros((T, P, P), dtype=np.float32)
        ohexp[tt, segl - wlo[tt // TPG], pp] = 1.0
        ohexp = ohexp.astype(ml_dtypes.bfloat16)

        gidx = np.full((P, NG), SMAX, dtype=np.int32)        # -> zero tail
        glen = np.zeros((P, NG), dtype=np.float32)
        obag = np.zeros((P, NG), dtype=np.int32)
        pr = np.arange(P)
        for g in range(NG):
            obag[:, g] = NB_LOC + pr                         # default: pad rows
            if whi[g] < 0:
                continue
            nreal = int(whi[g] - wlo[g]) + 1
            gb = b0 + wlo[g] + pr[:nreal]                    # global bag ids
            gidx[:nreal, g] = (starts[gb] - s0).astype(np.int32)
            glen[:nreal, g] = lens[gb].astype(np.float32)
            obag[:nreal, g] = (wlo[g] + pr[:nreal]).astype(np.int32)

        in_maps.append(
            dict(
                xpad=xpad,
                ohrel=ohrel,
                ohexp=ohexp,
                segloc=segloc,
                gidx=gidx,
                glen=glen,
                obag=obag,
                cons=cons_np,
                wt=wt_np,
            )
        )

    meta = dict(C=C, R=R, NB_LOC=NB_LOC, SMAX=SMAX, T=T, NG=NG, LW=LW)
    return meta, in_maps


# ----------------------------------------------------------------------------
# Bass program
# ----------------------------------------------------------------------------

def _build(meta):
    C, R = meta["C"], meta["R"]
    NB_LOC, SMAX, T, NG, LW = (
        meta["NB_LOC"], meta["SMAX"], meta["T"], meta["NG"], meta["LW"]
    )
    NBP = NB_LOC + P                                         # output rows incl pad
    SZ = SMAX + LW + P                                       # s_lin length
    NCH = C // P                                             # 8 c-chunks

    nc = bacc.Bacc("TRN2", target_bir_lowering=False, debug=False,
                   num_devices=NCORES)

    xpad = nc.dram_tensor("xpad", (SMAX, C), F32, kind="ExternalInput").ap()
    ohrel = nc.dram_tensor("ohrel", (T, R, P), F32, kind="ExternalInput").ap()
    ohexp = nc.dram_tensor("ohexp", (T, P, P), BF16, kind="ExternalInput").ap()
    segloc = nc.dram_tensor("segloc", (P, T), F32, kind="ExternalInput").ap()
    gidx = nc.dram_tensor("gidx", (P, NG), I32, kind="ExternalInput").ap()
    glen = nc.dram_tensor("glen", (P, NG), F32, kind="ExternalInput").ap()
    obag = nc.dram_tensor("obag", (P, NG), I32, kind="ExternalInput").ap()
    cons = nc.dram_tensor("cons", (R, C), F32, kind="ExternalInput").ap()
    wt = nc.dram_tensor("wt", (C, R), F32, kind="ExternalInput").ap()

    s_lin = nc.dram_tensor("s_lin", (1, SZ), F32, kind="Internal").ap()
    outc = nc.dram_tensor("outc", (NBP, R), F32, kind="ExternalOutput").ap()

    with tile.TileContext(nc) as tc:
        with ExitStack() as ctx:
            singles = ctx.enter_context(tc.tile_pool(name="singles", bufs=1))
            xin = ctx.enter_context(tc.tile_pool(name="xin", bufs=20))
            oh1p = ctx.enter_context(tc.tile_pool(name="oh1", bufs=20))
            oh2p = ctx.enter_context(tc.tile_pool(name="oh2", bufs=20))
            prodp = ctx.enter_context(tc.tile_pool(name="prod", bufs=2))
            ohpp = ctx.enter_context(tc.tile_pool(name="ohp", bufs=3))
            spp = ctx.enter_context(tc.tile_pool(name="spp", bufs=3))
            tiny = ctx.enter_context(tc.tile_pool(name="tiny", bufs=4))
            bagnp = ctx.enter_context(tc.tile_pool(name="bagn", bufs=2))
            bagtp = ctx.enter_context(tc.tile_pool(name="bagt", bufs=2))
            clsp = ctx.enter_context(tc.tile_pool(name="clsp", bufs=2))

            big_ps = ctx.enter_context(
                tc.tile_pool(name="big_ps", bufs=1, space="PSUM"))
            bagwin_ps = ctx.enter_context(
                tc.tile_pool(name="bagwin_ps", bufs=2, space="PSUM"))
            small_ps = ctx.enter_context(
                tc.tile_pool(name="small_ps", bufs=2, space="PSUM"))

            # ---------------- persistent tiles ----------------
            cons_sb = singles.tile([R, C], F32)
            nc.sync.dma_start(out=cons_sb[:], in_=cons[:])
            wt_sb = singles.tile([P, NCH, R], F32)
            nc.sync.dma_start(
                out=wt_sb[:], in_=wt.rearrange("(k p) r -> p k r", p=P))
            segloc_sb = singles.tile([P, T], F32)
            nc.sync.dma_start(out=segloc_sb[:], in_=segloc[:])
            gidx_sb = singles.tile([P, NG], I32)
            nc.sync.dma_start(out=gidx_sb[:], in_=gidx[:])
            glen_sb = singles.tile([P, NG], F32)
            nc.sync.dma_start(out=glen_sb[:], in_=glen[:])
            obag_sb = singles.tile([P, NG], I32)
            nc.sync.dma_start(out=obag_sb[:], in_=obag[:])

            identity = singles.tile([P, P], F32)
            make_identity(nc, identity[:])
            iota_sb = singles.tile([P, P], F32)
            nc.gpsimd.iota(iota_sb[:], [[1, P]], channel_multiplier=0,
                           allow_small_or_imprecise_dtypes=True)
            iota1_sb = singles.tile([P, LW], F32)
            nc.gpsimd.iota(iota1_sb[:], [[1, LW]], base=1, channel_multiplier=0,
                           allow_small_or_imprecise_dtypes=True)

            s_wide = singles.tile([P, T], F32)
            e_wide = singles.tile([P, T], F32)
            mnegbf = singles.tile([P, NG], BF16)
            mnegr = singles.tile([P, NG], F32)
            rw = singles.tile([P, NG], F32)
            ones_sb = singles.tile([P, 1], F32)
            nc.vector.memset(ones_sb[:], 1.0)

            ztail = singles.tile([1, SZ], F32)
            nc.vector.memset(ztail[:], 0.0)
            z_inst = nc.sync.dma_start(out=s_lin[:], in_=ztail[:])

            s_dma_insts = [[] for _ in range(NG)]
            prev_scatter = [None]

            # ---------------- phase 1: scores, one tile ----------------
            def phase1_tile(g, ti):
                t = g * TPG + ti
                ohr_t = oh1p.tile([R, P], F32, tag="ohr")
                nc.scalar.dma_start(out=ohr_t[:], in_=ohrel[t, :, :])
                ohe_t = oh2p.tile([P, P], BF16, tag="ohe")
                nc.scalar.dma_start(out=ohe_t[:], in_=ohexp[t, :, :])
                x_t = xin.tile([P, C], F32, tag="x")
                nc.sync.dma_start(out=x_t[:], in_=xpad[t * P:(t + 1) * P, :])

                conper = big_ps.tile([P, C], F32, tag="bigps")
                for h in range(2):
                    nc.tensor.matmul(
                        out=conper[:, h * 512:(h + 1) * 512],
                        lhsT=ohr_t[:],
                        rhs=cons_sb[:, h * 512:(h + 1) * 512],
                        start=True, stop=True)

                prod = prodp.tile([P, C], F32, tag="prod")
                nc.vector.tensor_tensor(
                    out=prod[:], in0=x_t[:], in1=conper[:],
                    op=mybir.AluOpType.mult)
                scr = prodp.tile([P, C], F32, tag="scr")
                nc.scalar.activation(
                    out=scr[:], in_=prod[:],
                    func=mybir.ActivationFunctionType.Copy,
                    accum_out=s_wide[:, t:t + 1])
                return x_t, ohe_t

            def phase1_tail(g):
                # s (4 tile columns) -> DRAM s_lin (sentence-linear layout)
                dst = s_lin[0:1, g * TPG * P:(g + 1) * TPG * P]
                di = nc.sync.dma_start(
                    out=dst.rearrange("o (t p) -> (o p) t", t=TPG),
                    in_=s_wide[:, g * TPG:(g + 1) * TPG])
                s_dma_insts[g].append(di)

            # ---------------- phase 2: per-bag max / denom for group g ----
            def phase2(g):
                sp = spp.tile([P, LW], F32, tag="sp")
                gi = nc.gpsimd.indirect_dma_start(
                    out=sp[:],
                    out_offset=None,
                    in_=s_lin[:],
                    in_offset=IndirectOffsetOnAxis(ap=gidx_sb[:, g:g + 1], axis=1),
                )
                for gg in (g, min(g + 1, NG - 1)):
                    for di in s_dma_insts[gg]:
                        add_dep_helper(gi.ins, di.ins, reason="s_lin RAW")
                add_dep_helper(gi.ins, z_inst.ins, reason="s_lin tail RAW")

                # masked per-bag max: amask = min((len - (j+1)) * 6e4, 0)
                d = spp.tile([P, LW], F32, tag="d")
                nc.vector.tensor_tensor(
                    out=d[:], in0=glen_sb[:, g:g + 1].to_broadcast([P, LW]),
                    in1=iota1_sb[:], op=mybir.AluOpType.subtract)
                am = spp.tile([P, LW], F32, tag="am")
                nc.vector.tensor_scalar(
                    out=am[:], in0=d[:], scalar1=60000.0, scalar2=0.0,
                    op0=mybir.AluOpType.mult, op1=mybir.AluOpType.min)
                spm = spp.tile([P, LW], F32, tag="spm")
                nc.vector.tensor_tensor(
                    out=spm[:], in0=sp[:], in1=am[:], op=mybir.AluOpType.add)
                mtmp = tiny.tile([P, 1], F32, tag="mtmp")
                nc.vector.tensor_reduce(
                    out=mtmp[:], in_=spm[:], axis=mybir.AxisListType.X,
                    op=mybir.AluOpType.max, negate=True)
                nc.vector.tensor_scalar(
                    out=mnegbf[:, g:g + 1], in0=mtmp[:], scalar1=300.0,
                    scalar2=None, op0=mybir.AluOpType.min)
                nc.vector.tensor_copy(out=mnegr[:, g:g + 1],
                                      in_=mnegbf[:, g:g + 1])
                epad = spp.tile([P, LW], F32, tag="epad")
                den = tiny.tile([P, 1], F32, tag="den")
                nc.scalar.activation(
                    out=epad[:], in_=spm[:],
                    func=mybir.ActivationFunctionType.Exp,
                    bias=mnegr[:, g:g + 1], scale=1.0, accum_out=den[:])
                dene = tiny.tile([P, 1], F32, tag="dene")
                nc.vector.tensor_scalar_add(dene[:], den[:], 1e-30)
                nc.vector.reciprocal(out=rw[:, g:g + 1], in_=dene[:])

            # ---------------- phase 3: pooling + classifier for group g ---
            def phase3_tile(g, ti, state, x_t, ohe_t):
                t = g * TPG + ti
                if ti == 0:
                    state["mcls"] = small_ps.tile([P, R + TPG], F32, tag="smallps", name="mcls")
                    state["bagwin"] = bagwin_ps.tile([P, C], F32, tag="bagwin", name="bagwin")
                mcls, bagwin = state["mcls"], state["bagwin"]
                nc.tensor.matmul(
                    out=mcls[:, R + ti:R + ti + 1],
                    lhsT=ohe_t[:],
                    rhs=mnegbf[:, g:g + 1],
                    start=True, stop=True)
                nc.scalar.activation(
                    out=e_wide[:, t:t + 1],
                    in_=mcls[:, R + ti:R + ti + 1],
                    func=mybir.ActivationFunctionType.Exp,
                    bias=s_wide[:, t:t + 1],
                    scale=1.0)
                ohp = ohpp.tile([P, P], F32, tag="ohp")
                nc.vector.tensor_scalar(
                    out=ohp[:],
                    in0=iota_sb[:],
                    scalar1=segloc_sb[:, t:t + 1],
                    scalar2=e_wide[:, t:t + 1],
                    op0=mybir.AluOpType.is_equal,
                    op1=mybir.AluOpType.mult)
                for h in range(2):
                    nc.tensor.matmul(
                        out=bagwin[:, h * 512:(h + 1) * 512],
                        lhsT=ohp[:],
                        rhs=x_t[:, h * 512:(h + 1) * 512],
                        start=(ti == 0), stop=(ti == TPG - 1))

            def phase3_tail_a(g, state):
                mcls, bagwin = state["mcls"], state["bagwin"]

                # normalize (fold rw) while copying PSUM -> SBUF
                bagn = bagnp.tile([P, C], F32, tag="bagn")
                nc.scalar.activation(
                    out=bagn[:],
                    in_=bagwin[:],
                    func=mybir.ActivationFunctionType.Copy,
                    scale=rw[:, g:g + 1])

                state["bagn"] = bagn

            def phase3_tail_b(g, state):
                bagn = state["bagn"]
                # transpose window: [bag, C] -> [C-chunk, bag] blocks
                bagt_ps = bagwin_ps.tile([P, C], F32, tag="bagwin")
                for cch in range(NCH):
                    nc.tensor.transpose(
                        out=bagt_ps[:, cch * P:(cch + 1) * P],
                        in_=bagn[:, cch * P:(cch + 1) * P],
                        identity=identity[:])
                bagt = bagtp.tile([P, C], F32, tag="bagt")
                nc.scalar.copy(out=bagt[:], in_=bagt_ps[:])
                state["bagt"] = bagt

            def phase3_tail_c(g, state):
                mcls, bagt = state["mcls"], state["bagt"]
                # classifier: out[bag, r] = sum_c bag[bag, c] * W[r, c]
                clsps = mcls[:, 0:R]
                for cch in range(NCH):
                    nc.tensor.matmul(
                        out=clsps,
                        lhsT=bagt[:, cch * P:(cch + 1) * P],
                        rhs=wt_sb[:, cch, :],
                        start=(cch == 0), stop=(cch == NCH - 1))
                cls_sb = clsp.tile([P, R], F32, tag="cls")
                nc.scalar.copy(out=cls_sb[:], in_=clsps)

                si = nc.gpsimd.indirect_dma_start(
                    out=outc[:],
                    out_offset=IndirectOffsetOnAxis(ap=obag_sb[:, g:g + 1], axis=0),
                    in_=cls_sb[:],
                    in_offset=None,
                    compute_op=mybir.AluOpType.add)
                if prev_scatter[0] is not None:
                    add_dep_helper(si.ins, prev_scatter[0].ins, reason="outc WAW")
                prev_scatter[0] = si

            # ---------------- pipeline ----------------
            LAG = 3 if NG > 3 else 2
            live = {}
            states = {}
            tailq = []          # queue of (fn, g) staggered work pieces

            def run_p3_tile(g, ti):
                x_t, ohe_t = live[g][ti]
                states.setdefault(g, {})
                phase3_tile(g, ti, states[g], x_t, ohe_t)

            def pop_tail():
                if tailq:
                    fn, gg = tailq.pop(0)
                    fn(gg, states[gg])

            for g in range(NG):
                tiles = []
                for ti in range(TPG):
                    tiles.append(phase1_tile(g, ti))
                    live[g] = tiles
                    if g >= LAG:
                        run_p3_tile(g - LAG, ti)
                    pop_tail()
                phase1_tail(g)
                if g >= LAG:
                    gg = g - LAG
                    tailq.append((phase3_tail_a, gg))
                    tailq.append((phase3_tail_b, gg))
                    tailq.append((phase3_tail_c, gg))
                if g >= 1:
                    phase2(g - 1)
            phase2(NG - 1)
            for g in range(max(0, NG - LAG), NG):
                for ti in range(TPG):
                    run_p3_tile(g, ti)
                    pop_tail()
                tailq.append((phase3_tail_a, g))
                tailq.append((phase3_tail_b, g))
                tailq.append((phase3_tail_c, g))
            while tailq:
                pop_tail()
            for gg in list(states):
                states.pop(gg)
            live.clear()

    nc.compile()
    return nc


_CACHE = {}


def _get_program(meta):
    key = tuple(sorted(meta.items()))
    if key not in _CACHE:
        _CACHE[key] = _build(meta)
    return _CACHE[key]


def kernel(X, Constraints, W, b, X_Scope, X_Rel):
    X = np.asarray(X)
    b_np = np.asarray(b, dtype=np.float32)
    meta, in_maps = _prep(X, Constraints, W, b, X_Scope, X_Rel)
    nc = _get_program(meta)
    res = run_bass_kernel_spmd(nc, in_maps, core_ids=list(range(NCORES)))
    NB_LOC = meta["NB_LOC"]
    parts = [res.results[k]["outc"][:NB_LOC] for k in range(NCORES)]
    out = np.concatenate(parts, axis=0) + b_np[None, :]
    return out.astype(np.float32)



# revision 2
# speedup vs baseline: 1.0015x; 1.0015x over previous
"""Trainium2 Bass kernel for CGRE-style ragged bag attention pooling + classifier.

Computation (per reference):
    seg[i]   : bag of sentence i (contiguous ragged scopes)
    s[i]     = X[i] . Constraints[X_Rel[seg[i]]]
    w[i]     = softmax of s within bag (numerically stabilized per bag)
    bag[b]   = sum_{i in b} w[i] * X[i]
    out      = bag @ W.T + b

Strategy: 8-way data parallel over contiguous bag ranges (1024 bags/core).
One shared SPMD Bass program; per-core raggedness is passed as data.

Per core, sentences run in 128-row tiles, 4 tiles per group; each group owns
a 128-bag window.  Pipeline per tile:
  - conper = onehot_rel.T @ Constraints   (fp32r matmul, exact row gather)
  - s      = rowsum(X * conper)           (one fused DVE op per 512-chunk)
Per group: scatter s to a sentence-linear DRAM line, gather per-bag score
rows, compute the stabilized per-bag softmax weights w, scatter w back to a
sentence-linear line.  Then pooling per tile:
  - bagwin += onehot_seg(w).T @ X         (fp32r matmul)
and per group: fp16 transpose of the pooled window + fp16 classifier matmul.
Group partial outputs go to DRAM contiguously; straddling bags are combined
on the host during unsharding (tiny [~2k,100] adds).

fp32r inputs (X, Constraints, one-hots) are pre-rounded on the host
(round-to-nearest, 11 mantissa bits) and declared as float32r DRAM tensors.
"""

import numpy as np
from contextlib import ExitStack

import ml_dtypes

import concourse.bass as bass
import concourse.tile as tile
from concourse import bacc, mybir
from concourse.bass import IndirectOffsetOnAxis
from concourse.bass_utils import run_bass_kernel_spmd
from concourse.masks import make_identity
from concourse.tile import add_dep_helper

NCORES = 8
P = 128
TPG = 4          # tiles per group (512 sentences)
LAG = 3          # groups between score production and pooling
F32 = mybir.dt.float32
F32R = mybir.dt.float32r
F16 = mybir.dt.float16
I32 = mybir.dt.int32


def _round_f32r(x):
    """Host emulation of TRN2 fp32r rounding (round-to-nearest, 11 mantissa
    bits) so float32r DRAM inputs match on-chip rounded operands."""
    u = np.ascontiguousarray(x, np.float32).view(np.uint32)
    half = np.uint32(1 << 11)
    mask = np.uint32(~np.uint32((1 << 12) - 1))
    return ((u + half) & mask).view(np.float32)


# ----------------------------------------------------------------------------
# Host-side preparation
# ----------------------------------------------------------------------------

def _prep(X, Constraints, W, b, X_Scope, X_Rel):
    N, C = X.shape
    R = Constraints.shape[0]
    B = X_Scope.shape[0]
    assert B % NCORES == 0
    NB_LOC = B // NCORES

    starts = np.asarray(X_Scope[:, 0], dtype=np.int64)
    ends = np.asarray(X_Scope[:, 1], dtype=np.int64)
    lens = ends - starts
    seg = np.searchsorted(starts, np.arange(N), side="right") - 1
    rel_s = np.asarray(X_Rel, dtype=np.int64)[seg]          # relation per sentence

    LW = int(max(64, ((lens.max() + 31) // 32) * 32))        # gather row width

    core_b0 = [k * NB_LOC for k in range(NCORES)]
    core_s0 = [int(starts[b0]) for b0 in core_b0]
    core_s1 = [int(ends[b0 + NB_LOC - 1]) for b0 in core_b0]
    cnts = [s1 - s0 for s0, s1 in zip(core_s0, core_s1)]

    GS = TPG * P
    SMAX = ((max(cnts) + GS - 1) // GS) * GS
    T = SMAX // P
    NG = T // TPG

    def win_ok(gs):
        for k in range(NCORES):
            s0, cnt = core_s0[k], cnts[k]
            for g0 in range(0, cnt, gs):
                lo = seg[s0 + g0]
                hi = seg[s0 + min(g0 + gs, cnt) - 1]
                if hi - lo + 1 > P:
                    return False
        return True

    assert win_ok(GS), (
        "128-bag window does not cover a 512-sentence group; "
        "scope distribution far from expected"
    )

    X = np.ascontiguousarray(np.asarray(X, dtype=np.float32))
    cons_r = _round_f32r(np.asarray(Constraints, dtype=np.float32))
    wt16 = np.zeros((P, (C // P) * R), dtype=np.float16)
    Wt = np.asarray(W, dtype=np.float32)                     # [R, C]
    for k in range(C // P):
        wt16[:, k * R:(k + 1) * R] = Wt[:, k * P:(k + 1) * P].T.astype(np.float16)

    in_maps = []
    combine = []                                             # host unshard info
    for k in range(NCORES):
        s0, s1, cnt = core_s0[k], core_s1[k], cnts[k]
        b0 = core_b0[k]

        xpad = np.zeros((SMAX, C), dtype=np.float32)
        xpad[:cnt] = X[s0:s1]
        xpad = _round_f32r(xpad)

        jj = np.arange(cnt)
        tt = jj // P
        pp = jj % P
        segl = seg[s0:s1] - b0                               # local bag ids

        wlo = np.zeros(NG, dtype=np.int64)
        whi = np.full(NG, -1, dtype=np.int64)
        for g in range(NG):
            g0 = g * GS
            if g0 < cnt:
                wlo[g] = segl[g0]
                whi[g] = segl[min(g0 + GS, cnt) - 1]

        segloc = np.full((P, T), -1.0, dtype=np.float32)
        segloc[pp, tt] = (segl - wlo[tt // TPG]).astype(np.float32)

        ohr = np.zeros((R, SMAX), dtype=np.float32)          # one-hot, exact
        ohr[rel_s[s0:s1], jj] = 1.0

        gidx = np.full((P, NG), SMAX, dtype=np.int32)        # -> zero tail
        glen = np.zeros((P, NG), dtype=np.float32)
        pr = np.arange(P)
        for g in range(NG):
            if whi[g] < 0:
                continue
            nreal = int(whi[g] - wlo[g]) + 1
            gb = b0 + wlo[g] + pr[:nreal]                    # global bag ids
            gidx[:nreal, g] = (starts[gb] - s0).astype(np.int32)
            glen[:nreal, g] = lens[gb].astype(np.float32)

        in_maps.append(
            dict(
                xpad=xpad,
                ohr=ohr,
                segloc=segloc,
                gidx=gidx,
                glen=glen,
                cons=cons_r,
                wt16=wt16,
            )
        )
        combine.append(dict(b0=b0, wlo=wlo, whi=whi))

    meta = dict(C=C, R=R, NB_LOC=NB_LOC, SMAX=SMAX, T=T, NG=NG, LW=LW)
    return meta, in_maps, combine


# ----------------------------------------------------------------------------
# Bass program
# ----------------------------------------------------------------------------

def _build(meta):
    C, R = meta["C"], meta["R"]
    SMAX, T, NG, LW = meta["SMAX"], meta["T"], meta["NG"], meta["LW"]
    SZ = SMAX + LW + P
    NCH = C // P

    nc = bacc.Bacc("TRN2", target_bir_lowering=False, debug=False,
                   num_devices=NCORES)

    xpad = nc.dram_tensor("xpad", (SMAX, C), F32R, kind="ExternalInput").ap()
    ohr = nc.dram_tensor("ohr", (R, SMAX), F32R, kind="ExternalInput").ap()
    cons = nc.dram_tensor("cons", (R, C), F32R, kind="ExternalInput").ap()
    wt16 = nc.dram_tensor("wt16", (P, NCH * R), F16, kind="ExternalInput").ap()
    segloc = nc.dram_tensor("segloc", (P, T), F32, kind="ExternalInput").ap()
    gidx = nc.dram_tensor("gidx", (P, NG), I32, kind="ExternalInput").ap()
    glen = nc.dram_tensor("glen", (P, NG), F32, kind="ExternalInput").ap()

    s_lin = nc.dram_tensor("s_lin", (1, SZ), F32, kind="Internal").ap()
    w_lin = nc.dram_tensor("w_lin", (1, SZ), F32, kind="Internal").ap()
    outg = nc.dram_tensor("outg", (NG * P, R), F32, kind="ExternalOutput").ap()

    with tile.TileContext(nc) as tc:
        with ExitStack() as ctx:
            singles = ctx.enter_context(tc.tile_pool(name="singles", bufs=1))
            xin = ctx.enter_context(tc.tile_pool(name="xin", bufs=18))
            prodp = ctx.enter_context(tc.tile_pool(name="prod", bufs=2))
            ohpp = ctx.enter_context(tc.tile_pool(name="ohp", bufs=3))
            spp = ctx.enter_context(tc.tile_pool(name="spp", bufs=2))
            tiny = ctx.enter_context(tc.tile_pool(name="tiny", bufs=3))
            bagnp = ctx.enter_context(tc.tile_pool(name="bagn", bufs=2))
            bagtp = ctx.enter_context(tc.tile_pool(name="bagt", bufs=2))
            clsp = ctx.enter_context(tc.tile_pool(name="clsp", bufs=2))

            conper_ps = ctx.enter_context(
                tc.tile_pool(name="conper_ps", bufs=2, space="PSUM"))
            bagwin_ps = ctx.enter_context(
                tc.tile_pool(name="bagwin_ps", bufs=2, space="PSUM"))
            bagt_ps = ctx.enter_context(
                tc.tile_pool(name="bagt_ps", bufs=1, space="PSUM"))
            mcls_ps = ctx.enter_context(
                tc.tile_pool(name="mcls_ps", bufs=1, space="PSUM"))

            # ---------------- persistent tiles ----------------
            ohr_sb = singles.tile([R, SMAX], F32R)
            nc.sync.dma_start(out=ohr_sb[:], in_=ohr[:])
            cons_sb = singles.tile([R, C], F32R)
            nc.sync.dma_start(out=cons_sb[:], in_=cons[:])
            wt_sb = singles.tile([P, NCH * R], F16)
            nc.sync.dma_start(out=wt_sb[:], in_=wt16[:])
            segloc_sb = singles.tile([P, T], F32)
            nc.sync.dma_start(out=segloc_sb[:], in_=segloc[:])
            gidx_sb = singles.tile([P, NG], I32)
            nc.sync.dma_start(out=gidx_sb[:], in_=gidx[:])
            glen_sb = singles.tile([P, NG], F32)
            nc.sync.dma_start(out=glen_sb[:], in_=glen[:])

            ident16 = singles.tile([P, P], F16)
            make_identity(nc, ident16[:])
            iota_sb = singles.tile([P, P], F32)
            nc.gpsimd.iota(iota_sb[:], [[1, P]], channel_multiplier=0,
                           allow_small_or_imprecise_dtypes=True)
            iota1_sb = singles.tile([P, LW], F32)
            nc.gpsimd.iota(iota1_sb[:], [[1, LW]], base=1, channel_multiplier=0,
                           allow_small_or_imprecise_dtypes=True)

            s_wide = singles.tile([P, T], F32)
            w_wide = singles.tile([P, T], F32)

            ztail = singles.tile([1, SZ], F32)
            nc.vector.memset(ztail[:], 0.0)
            z_s = nc.sync.dma_start(out=s_lin[:], in_=ztail[:])
            z_w = nc.sync.dma_start(out=w_lin[:], in_=ztail[:])

            s_dma_insts = [[] for _ in range(NG)]
            w_scatter_insts = [None] * NG
            live = {}

            # ---------------- phase 1: scores, one tile ----------------
            def phase1_tile(t):
                x_t = xin.tile([P, C], F32R, tag="x")
                nc.sync.dma_start(out=x_t[:], in_=xpad[t * P:(t + 1) * P, :])
                sh = []
                for h in range(2):
                    cp = conper_ps.tile([P, 512], F32, tag="cp", name="cp")
                    nc.tensor.matmul(
                        out=cp[:],
                        lhsT=ohr_sb[:, t * P:(t + 1) * P],
                        rhs=cons_sb[:, h * 512:(h + 1) * 512],
                        start=True, stop=True)
                    prod = prodp.tile([P, 512], F32, tag="prod")
                    acc = tiny.tile([P, 1], F32, tag=f"sh{h}")
                    nc.vector.scalar_tensor_tensor(
                        out=prod[:],
                        in0=x_t[:, h * 512:(h + 1) * 512].bitcast(F32),
                        scalar=1.0,
                        in1=cp[:],
                        op0=mybir.AluOpType.mult,
                        op1=mybir.AluOpType.mult,
                        accum_out=acc[:])
                    sh.append(acc)
                nc.vector.tensor_tensor(
                    out=s_wide[:, t:t + 1], in0=sh[0][:], in1=sh[1][:],
                    op=mybir.AluOpType.add)
                live[t] = x_t

            def phase1_tail(g):
                dst = s_lin[0:1, g * TPG * P:(g + 1) * TPG * P]
                di = nc.sync.dma_start(
                    out=dst.rearrange("o (t p) -> (o p) t", t=TPG),
                    in_=s_wide[:, g * TPG:(g + 1) * TPG])
                s_dma_insts[g].append(di)

            # ---------------- phase 2: per-bag softmax weights ----------
            def phase2(g):
                sp = spp.tile([P, LW], F32, tag="sp")
                gi = nc.gpsimd.indirect_dma_start(
                    out=sp[:],
                    out_offset=None,
                    in_=s_lin[:],
                    in_offset=IndirectOffsetOnAxis(ap=gidx_sb[:, g:g + 1], axis=1),
                )
                for gg in (g, min(g + 1, NG - 1)):
                    for di in s_dma_insts[gg]:
                        add_dep_helper(gi.ins, di.ins, reason="s_lin RAW")
                add_dep_helper(gi.ins, z_s.ins, reason="s_lin tail RAW")

                # masked per-bag max: amask = min((len - (j+1)) * 6e4, 0)
                d = spp.tile([P, LW], F32, tag="d")
                nc.vector.tensor_tensor(
                    out=d[:], in0=glen_sb[:, g:g + 1].to_broadcast([P, LW]),
                    in1=iota1_sb[:], op=mybir.AluOpType.subtract)
                am = spp.tile([P, LW], F32, tag="am")
                nc.vector.tensor_scalar(
                    out=am[:], in0=d[:], scalar1=60000.0, scalar2=0.0,
                    op0=mybir.AluOpType.mult, op1=mybir.AluOpType.min)
                spm = spp.tile([P, LW], F32, tag="spm")
                nc.vector.tensor_tensor(
                    out=spm[:], in0=sp[:], in1=am[:], op=mybir.AluOpType.add)
                mtmp = tiny.tile([P, 1], F32, tag="mtmp")
                nc.vector.tensor_reduce(
                    out=mtmp[:], in_=spm[:], axis=mybir.AxisListType.X,
                    op=mybir.AluOpType.max, negate=True)
                mneg = tiny.tile([P, 1], F32, tag="mneg")
                nc.vector.tensor_scalar(
                    out=mneg[:], in0=mtmp[:], scalar1=300.0,
                    scalar2=None, op0=mybir.AluOpType.min)
                epad = spp.tile([P, LW], F32, tag="epad")
                den = tiny.tile([P, 1], F32, tag="den")
                nc.scalar.activation(
                    out=epad[:], in_=spm[:],
                    func=mybir.ActivationFunctionType.Exp,
                    bias=mneg[:], scale=1.0, accum_out=den[:])
                dene = tiny.tile([P, 1], F32, tag="dene")
                nc.vector.tensor_scalar_add(dene[:], den[:], 1e-30)
                rw = tiny.tile([P, 1], F32, tag="rw")
                nc.vector.reciprocal(out=rw[:], in_=dene[:])
                wrow = spp.tile([P, LW], F32, tag="wrow")
                nc.vector.tensor_scalar(
                    out=wrow[:], in0=epad[:], scalar1=rw[:], scalar2=None,
                    op0=mybir.AluOpType.mult)
                si = nc.gpsimd.indirect_dma_start(
                    out=w_lin[:],
                    out_offset=IndirectOffsetOnAxis(ap=gidx_sb[:, g:g + 1], axis=1),
                    in_=wrow[:],
                    in_offset=None,
                )
                add_dep_helper(si.ins, z_w.ins, reason="w_lin tail WAW")
                w_scatter_insts[g] = si

            # ---------------- phase 3: pooling + classifier -------------
            def phase3_pre(g):
                src = w_lin[0:1, g * TPG * P:(g + 1) * TPG * P]
                wi = nc.sync.dma_start(
                    out=w_wide[:, g * TPG:(g + 1) * TPG],
                    in_=src.rearrange("o (t p) -> (o p) t", t=TPG))
                add_dep_helper(wi.ins, w_scatter_insts[g].ins, reason="w RAW")
                add_dep_helper(wi.ins, z_w.ins, reason="w_lin tail RAW")

            def phase3_tile(g, ti, state):
                t = g * TPG + ti
                x_t = live.pop(t)
                ohp = ohpp.tile([P, P], F32R, tag="ohp")
                nc.vector.tensor_scalar(
                    out=ohp[:],
                    in0=iota_sb[:],
                    scalar1=segloc_sb[:, t:t + 1],
                    scalar2=w_wide[:, t:t + 1],
                    op0=mybir.AluOpType.is_equal,
                    op1=mybir.AluOpType.mult)
                if ti == 0:
                    state["bagwin"] = bagwin_ps.tile(
                        [P, C], F32, tag="bagwin", name="bagwin")
                bagwin = state["bagwin"]
                for h in range(2):
                    nc.tensor.matmul(
                        out=bagwin[:, h * 512:(h + 1) * 512],
                        lhsT=ohp[:],
                        rhs=x_t[:, h * 512:(h + 1) * 512],
                        start=(ti == 0), stop=(ti == TPG - 1))

            def phase3_tail_a(g, state):
                bagn16 = bagnp.tile([P, C], F16, tag="bagn16")
                nc.scalar.activation(
                    out=bagn16[:], in_=state["bagwin"][:],
                    func=mybir.ActivationFunctionType.Copy)
                state["bagn16"] = bagn16

            def phase3_tail_b(g, state):
                btp = bagt_ps.tile([P, C], F16, tag="bagtps", name="bagtps")
                bagn16 = state["bagn16"]
                for cch in range(NCH):
                    nc.tensor.transpose(
                        out=btp[:, cch * P:(cch + 1) * P],
                        in_=bagn16[:, cch * P:(cch + 1) * P],
                        identity=ident16[:])
                bagt = bagtp.tile([P, C], F16, tag="bagt")
                nc.scalar.copy(out=bagt[:], in_=btp[:])
                state["bagt"] = bagt

            def phase3_tail_c(g, state):
                mcls = mcls_ps.tile([P, R], F32, tag="mcls", name="mcls")
                bagt = state["bagt"]
                for cch in range(NCH):
                    nc.tensor.matmul(
                        out=mcls[:],
                        lhsT=bagt[:, cch * P:(cch + 1) * P],
                        rhs=wt_sb[:, cch * R:(cch + 1) * R],
                        start=(cch == 0), stop=(cch == NCH - 1))
                cls_sb = clsp.tile([P, R], F32, tag="cls")
                nc.scalar.copy(out=cls_sb[:], in_=mcls[:])
                nc.sync.dma_start(
                    out=outg[g * P:(g + 1) * P, :], in_=cls_sb[:])

            # ---------------- pipeline ----------------
            states = {}
            tailq = []

            def pop_tail():
                if tailq:
                    fn, gg = tailq.pop(0)
                    fn(gg, states[gg])

            for g in range(NG):
                for ti in range(TPG):
                    phase1_tile(g * TPG + ti)
                    if g >= LAG:
                        gg = g - LAG
                        if ti == 0:
                            phase3_pre(gg)
                            states[gg] = {}
                        phase3_tile(gg, ti, states[gg])
                    pop_tail()
                phase1_tail(g)
                if g >= 1:
                    phase2(g - 1)
                if g >= LAG:
                    gg = g - LAG
                    tailq.append((phase3_tail_a, gg))
                    tailq.append((phase3_tail_b, gg))
                    tailq.append((phase3_tail_c, gg))
            phase2(NG - 1)
            for g in range(max(0, NG - LAG), NG):
                phase3_pre(g)
                states[g] = {}
                for ti in range(TPG):
                    phase3_tile(g, ti, states[g])
                    pop_tail()
                tailq.append((phase3_tail_a, g))
                tailq.append((phase3_tail_b, g))
                tailq.append((phase3_tail_c, g))
            while tailq:
                pop_tail()
            states.clear()
            live.clear()

    nc.compile()
    return nc


_CACHE = {}


def _get_program(meta):
    key = tuple(sorted(meta.items()))
    if key not in _CACHE:
        _CACHE[key] = _build(meta)
    return _CACHE[key]


def kernel(X, Constraints, W, b, X_Scope, X_Rel):
    X = np.asarray(X)
    b_np = np.asarray(b, dtype=np.float32)
    meta, in_maps, combine = _prep(X, Constraints, W, b, X_Scope, X_Rel)
    nc = _get_program(meta)
    res = run_bass_kernel_spmd(nc, in_maps, core_ids=list(range(NCORES)))
    B = X_Scope.shape[0]
    R = Constraints.shape[0]
    NG = meta["NG"]
    out = np.zeros((B, R), dtype=np.float32)
    for k in range(NCORES):
        og = res.results[k]["outg"]
        cb = combine[k]
        b0, wlo, whi = cb["b0"], cb["wlo"], cb["whi"]
        for g in range(NG):
            if whi[g] < 0:
                continue
            nreal = int(whi[g] - wlo[g]) + 1
            lo = b0 + int(wlo[g])
            out[lo:lo + nreal] += og[g * P:g * P + nreal]
    return out + b_np[None, :]


# revision 7
# speedup vs baseline: 1.4219x; 1.4197x over previous
"""Trainium2 Bass kernel for CGRE-style ragged bag attention pooling + classifier.

Computation (per reference):
    seg[i]   : bag of sentence i (contiguous ragged scopes)
    s[i]     = X[i] . Constraints[X_Rel[seg[i]]]
    w[i]     = softmax of s within bag (numerically stabilized per bag)
    bag[b]   = sum_{i in b} w[i] * X[i]
    out      = bag @ W.T + b

Strategy: 8-way data parallel over contiguous bag ranges (1024 bags/core).
One shared SPMD Bass program; per-core raggedness is passed as data.

Per core, sentences run in 128-row tiles, 4 tiles per group; each group owns
a 128-bag window.  Per tile:
  - conper = onehot_rel.T @ Constraints   (fp32r matmul, exact row gather)
  - s      = rowsum(X * conper)           (one fused DVE op)
Per group: scatter s to a sentence-linear DRAM line, gather per-bag score
rows, compute per-bag -max (bf16) and 1/denominator (kept in SBUF).  Pooling
per tile (lagged by LAG groups):
  - mneg_s = onehot_seg.T @ mneg          (tiny bf16 matmul broadcast)
  - e      = exp(s + mneg_s)              (scalar engine)
  - bagwin += onehot_seg(e).T @ X         (fp32r matmul, unnormalized)
Per group: fp16 transpose of the pooled window, fp16 classifier matmul, and
the 1/denominator is folded into the per-partition-scaled output copy.
Group partial outputs accumulate in SBUF and stream out in large chunks;
straddling bags are combined on the host during unsharding.

fp32r inputs (X, Constraints, rel one-hot) are pre-rounded on the host
(round-to-nearest, 11 mantissa bits) and declared float32r in DRAM.
"""

import numpy as np
from contextlib import ExitStack

import ml_dtypes

import concourse.bass as bass
import concourse.tile as tile
from concourse import bacc, mybir
from concourse.bass import IndirectOffsetOnAxis
from concourse.bass_utils import run_bass_kernel_spmd
from concourse.masks import make_identity
from concourse.tile import add_dep_helper

NCORES = 8
P = 128
TPG = 4          # tiles per group (512 sentences)
LAG = 3          # groups between score production and pooling
F32 = mybir.dt.float32
F32R = mybir.dt.float32r
F16 = mybir.dt.float16
BF16 = mybir.dt.bfloat16
I32 = mybir.dt.int32


def _round_f32r(x):
    """Host emulation of TRN2 fp32r rounding (round-to-nearest, 11 mantissa
    bits) so float32r DRAM inputs match on-chip rounded operands."""
    u = np.ascontiguousarray(x, np.float32).view(np.uint32)
    half = np.uint32(1 << 11)
    mask = np.uint32(~np.uint32((1 << 12) - 1))
    return ((u + half) & mask).view(np.float32)


# ----------------------------------------------------------------------------
# Host-side preparation
# ----------------------------------------------------------------------------

def _prep(X, Constraints, W, b, X_Scope, X_Rel):
    N, C = X.shape
    R = Constraints.shape[0]
    B = X_Scope.shape[0]
    assert B % NCORES == 0
    NB_LOC = B // NCORES

    starts = np.asarray(X_Scope[:, 0], dtype=np.int64)
    ends = np.asarray(X_Scope[:, 1], dtype=np.int64)
    lens = ends - starts
    seg = np.searchsorted(starts, np.arange(N), side="right") - 1
    rel_s = np.asarray(X_Rel, dtype=np.int64)[seg]          # relation per sentence

    LW = int(max(64, ((lens.max() + 31) // 32) * 32))        # gather row width

    core_b0 = [k * NB_LOC for k in range(NCORES)]
    core_s0 = [int(starts[b0]) for b0 in core_b0]
    core_s1 = [int(ends[b0 + NB_LOC - 1]) for b0 in core_b0]
    cnts = [s1 - s0 for s0, s1 in zip(core_s0, core_s1)]

    GS = TPG * P
    SMAX = ((max(cnts) + GS - 1) // GS) * GS
    T = SMAX // P
    NG = T // TPG

    def win_ok(gs):
        for k in range(NCORES):
            s0, cnt = core_s0[k], cnts[k]
            for g0 in range(0, cnt, gs):
                lo = seg[s0 + g0]
                hi = seg[s0 + min(g0 + gs, cnt) - 1]
                if hi - lo + 1 > P:
                    return False
        return True

    assert win_ok(GS), (
        "128-bag window does not cover a 512-sentence group; "
        "scope distribution far from expected"
    )

    X = np.ascontiguousarray(np.asarray(X, dtype=np.float32))
    cons_r = _round_f32r(np.asarray(Constraints, dtype=np.float32))
    wt16 = np.zeros((P, (C // P) * R), dtype=np.float16)
    Wt = np.asarray(W, dtype=np.float32)                     # [R, C]
    for k in range(C // P):
        wt16[:, k * R:(k + 1) * R] = Wt[:, k * P:(k + 1) * P].T.astype(np.float16)

    in_maps = []
    combine = []                                             # host unshard info
    for k in range(NCORES):
        s0, s1, cnt = core_s0[k], core_s1[k], cnts[k]
        b0 = core_b0[k]

        xpad = np.zeros((SMAX, C), dtype=np.float32)
        xpad[:cnt] = X[s0:s1]
        xpad = _round_f32r(xpad)

        jj = np.arange(cnt)
        tt = jj // P
        pp = jj % P
        segl = seg[s0:s1] - b0                               # local bag ids

        wlo = np.zeros(NG, dtype=np.int64)
        whi = np.full(NG, -1, dtype=np.int64)
        for g in range(NG):
            g0 = g * GS
            if g0 < cnt:
                wlo[g] = segl[g0]
                whi[g] = segl[min(g0 + GS, cnt) - 1]

        slotl = (segl - wlo[tt // TPG]).astype(np.int64)     # window slot per sent
        segloc = np.full((P, T), -1.0, dtype=np.float32)
        segloc[pp, tt] = slotl.astype(np.float32)

        ohr = np.zeros((R, SMAX), dtype=np.float32)          # rel one-hot, exact
        ohr[rel_s[s0:s1], jj] = 1.0

        ohe = np.zeros((P, T * P), dtype=ml_dtypes.bfloat16)  # slot one-hot
        ohe[slotl, jj] = 1.0

        gidx = np.full((P, NG), SMAX, dtype=np.int32)        # -> zero tail
        glen = np.zeros((P, NG), dtype=np.float32)
        pr = np.arange(P)
        for g in range(NG):
            if whi[g] < 0:
                continue
            nreal = int(whi[g] - wlo[g]) + 1
            gb = b0 + wlo[g] + pr[:nreal]                    # global bag ids
            gidx[:nreal, g] = (starts[gb] - s0).astype(np.int32)
            glen[:nreal, g] = lens[gb].astype(np.float32)

        in_maps.append(
            dict(
                xpad=xpad,
                ohr=ohr,
                ohe=ohe,
                segloc=segloc,
                gidx=gidx,
                glen=glen,
                cons=cons_r,
                wt16=wt16,
            )
        )
        combine.append(dict(b0=b0, wlo=wlo, whi=whi))

    meta = dict(C=C, R=R, NB_LOC=NB_LOC, SMAX=SMAX, T=T, NG=NG, LW=LW)
    return meta, in_maps, combine


# ----------------------------------------------------------------------------
# Bass program
# ----------------------------------------------------------------------------

def _build(meta):
    C, R = meta["C"], meta["R"]
    SMAX, T, NG, LW = meta["SMAX"], meta["T"], meta["NG"], meta["LW"]
    NCH = C // P

    nc = bacc.Bacc("TRN2", target_bir_lowering=False, debug=False,
                   num_devices=NCORES)

    xpad = nc.dram_tensor("xpad", (SMAX, C), F32R, kind="ExternalInput").ap()
    ohr = nc.dram_tensor("ohr", (R, SMAX), F32R, kind="ExternalInput").ap()
    ohe = nc.dram_tensor("ohe", (P, T * P), BF16, kind="ExternalInput").ap()
    cons = nc.dram_tensor("cons", (R, C), F32R, kind="ExternalInput").ap()
    wt16 = nc.dram_tensor("wt16", (P, NCH * R), F16, kind="ExternalInput").ap()
    segloc = nc.dram_tensor("segloc", (P, T), F32, kind="ExternalInput").ap()
    gidx = nc.dram_tensor("gidx", (P, NG), I32, kind="ExternalInput").ap()
    glen = nc.dram_tensor("glen", (P, NG), F32, kind="ExternalInput").ap()

    SZ = SMAX + ((LW + P - 1) // P) * P + P                  # multiple of P
    s_lin = nc.dram_tensor("s_lin", (1, SZ), F32, kind="Internal").ap()
    outg = nc.dram_tensor("outg", (P, NG * R), F32, kind="ExternalOutput").ap()

    OCH = 4                                                   # outg chunks
    GPC = (NG + OCH - 1) // OCH                               # groups per chunk

    with tile.TileContext(nc) as tc:
        with ExitStack() as ctx:
            singles = ctx.enter_context(tc.tile_pool(name="singles", bufs=1))
            xin = ctx.enter_context(tc.tile_pool(name="xin", bufs=18))
            prodp = ctx.enter_context(tc.tile_pool(name="prod", bufs=2))
            ohpp = ctx.enter_context(tc.tile_pool(name="ohp", bufs=3))
            spp = ctx.enter_context(tc.tile_pool(name="spp", bufs=2))
            tiny = ctx.enter_context(tc.tile_pool(name="tiny", bufs=4))
            bagnp = ctx.enter_context(tc.tile_pool(name="bagn", bufs=2))
            bagtp = ctx.enter_context(tc.tile_pool(name="bagt", bufs=2))

            conper_ps = ctx.enter_context(
                tc.tile_pool(name="conper_ps", bufs=2, space="PSUM"))
            bagwin_ps = ctx.enter_context(
                tc.tile_pool(name="bagwin_ps", bufs=1, space="PSUM"))
            bagt_ps = ctx.enter_context(
                tc.tile_pool(name="bagt_ps", bufs=1, space="PSUM"))
            mne_ps = ctx.enter_context(
                tc.tile_pool(name="mne_ps", bufs=1, space="PSUM"))

            # ---------------- persistent tiles ----------------
            def chunked_load(dst, src, width, nch):
                step = (width + nch - 1) // nch
                step = (step + P - 1) // P * P
                for c0 in range(0, width, step):
                    c1 = min(c0 + step, width)
                    nc.sync.dma_start(out=dst[:, c0:c1], in_=src[:, c0:c1])

            ohr_sb = singles.tile([R, SMAX], F32R)
            chunked_load(ohr_sb, ohr, SMAX, 8)
            ohe_sb = singles.tile([P, T * P], BF16)
            chunked_load(ohe_sb, ohe, T * P, 8)
            cons_sb = singles.tile([R, C], F32R)
            nc.sync.dma_start(out=cons_sb[:], in_=cons[:])
            wt_sb = singles.tile([P, NCH * R], F16)
            nc.sync.dma_start(out=wt_sb[:], in_=wt16[:])
            segloc_sb = singles.tile([P, T], F32)
            nc.sync.dma_start(out=segloc_sb[:], in_=segloc[:])
            gidx_sb = singles.tile([P, NG], I32)
            nc.sync.dma_start(out=gidx_sb[:], in_=gidx[:])
            glen_sb = singles.tile([P, NG], F32)
            nc.sync.dma_start(out=glen_sb[:], in_=glen[:])

            ident16 = singles.tile([P, P], F16)
            make_identity(nc, ident16[:])
            iota_sb = singles.tile([P, P], F32)
            nc.gpsimd.iota(iota_sb[:], [[1, P]], channel_multiplier=0,
                           allow_small_or_imprecise_dtypes=True)
            iota1_sb = singles.tile([P, LW], F32)
            nc.gpsimd.iota(iota1_sb[:], [[1, LW]], base=1, channel_multiplier=0,
                           allow_small_or_imprecise_dtypes=True)

            s_wide = singles.tile([P, T], F32)
            mnegb_w = singles.tile([P, NG], BF16)
            rw_w = singles.tile([P, NG], F32)
            cls_acc = singles.tile([P, NG * R], F32)

            ztail = singles.tile([P, SZ // P], F32)
            nc.vector.memset(ztail[:], 0.0)
            z_s = nc.sync.dma_start(
                out=s_lin.rearrange("o (p k) -> (o p) k", p=P), in_=ztail[:])

            s_batch = {}                                      # g -> dma inst
            phase2_done = [False] * NG
            live = {}

            # ---------------- phase 1: scores, one tile ----------------
            def phase1_tile(t):
                x_t = xin.tile([P, C], F32R, tag="x")
                for q in range(4):
                    nc.sync.dma_start(
                        out=x_t[q * 32:(q + 1) * 32, :],
                        in_=xpad[t * P + q * 32:t * P + (q + 1) * 32, :])
                cp = conper_ps.tile([P, C], F32, tag="cp", name="cp")
                for h in range(2):
                    nc.tensor.matmul(
                        out=cp[:, h * 512:(h + 1) * 512],
                        lhsT=ohr_sb[:, t * P:(t + 1) * P],
                        rhs=cons_sb[:, h * 512:(h + 1) * 512],
                        start=True, stop=True)
                prod = prodp.tile([P, C], F32, tag="prod")
                nc.vector.scalar_tensor_tensor(
                    out=prod[:],
                    in0=x_t[:].bitcast(F32),
                    scalar=1.0,
                    in1=cp[:],
                    op0=mybir.AluOpType.mult,
                    op1=mybir.AluOpType.mult,
                    accum_out=s_wide[:, t:t + 1])
                live[t] = x_t

            def s_flush(g0, g1):
                """DMA s_wide columns for groups [g0, g1] to s_lin."""
                ncols = (g1 - g0 + 1) * TPG
                dst = s_lin[0:1, g0 * TPG * P:(g1 + 1) * TPG * P]
                di = nc.sync.dma_start(
                    out=dst.rearrange("o (t p) -> (o p) t", t=ncols),
                    in_=s_wide[:, g0 * TPG:(g1 + 1) * TPG])
                for g in range(g0, g1 + 1):
                    s_batch[g] = di

            # ---------------- phase 2: per-bag -max and 1/denom ---------
            def phase2(g):
                sp = spp.tile([P, LW], F32, tag="sp")
                gi = nc.gpsimd.indirect_dma_start(
                    out=sp[:],
                    out_offset=None,
                    in_=s_lin[:],
                    in_offset=IndirectOffsetOnAxis(ap=gidx_sb[:, g:g + 1], axis=1),
                )
                for gg in (g, min(g + 1, NG - 1)):
                    add_dep_helper(gi.ins, s_batch[gg].ins, reason="s_lin RAW")
                add_dep_helper(gi.ins, z_s.ins, reason="s_lin tail RAW")

                # masked per-bag max: amask = min((len - (j+1)) * 6e4, 0)
                d = spp.tile([P, LW], F32, tag="d")
                nc.vector.tensor_tensor(
                    out=d[:], in0=glen_sb[:, g:g + 1].to_broadcast([P, LW]),
                    in1=iota1_sb[:], op=mybir.AluOpType.subtract)
                am = spp.tile([P, LW], F32, tag="am")
                nc.vector.tensor_scalar(
                    out=am[:], in0=d[:], scalar1=60000.0, scalar2=0.0,
                    op0=mybir.AluOpType.mult, op1=mybir.AluOpType.min)
                spm = spp.tile([P, LW], F32, tag="spm")
                nc.vector.tensor_tensor(
                    out=spm[:], in0=sp[:], in1=am[:], op=mybir.AluOpType.add)
                mtmp = tiny.tile([P, 1], F32, tag="mtmp")
                nc.vector.tensor_reduce(
                    out=mtmp[:], in_=spm[:], axis=mybir.AxisListType.X,
                    op=mybir.AluOpType.max, negate=True)
                nc.vector.tensor_scalar(
                    out=mnegb_w[:, g:g + 1], in0=mtmp[:], scalar1=300.0,
                    scalar2=None, op0=mybir.AluOpType.min)
                mnegf = tiny.tile([P, 1], F32, tag="mnegf")
                nc.vector.tensor_copy(out=mnegf[:], in_=mnegb_w[:, g:g + 1])
                epad = spp.tile([P, LW], F32, tag="epad")
                den = tiny.tile([P, 1], F32, tag="den")
                nc.scalar.activation(
                    out=epad[:], in_=spm[:],
                    func=mybir.ActivationFunctionType.Exp,
                    bias=mnegf[:], scale=1.0, accum_out=den[:])
                dene = tiny.tile([P, 1], F32, tag="dene")
                nc.vector.tensor_scalar_add(dene[:], den[:], 1e-30)
                nc.vector.reciprocal(out=rw_w[:, g:g + 1], in_=dene[:])
                phase2_done[g] = True

            # ---------------- phase 3: pooling + classifier -------------
            def phase3_head(g, ti, state):
                t = g * TPG + ti
                if ti == 0:
                    state["mne"] = mne_ps.tile([P, TPG], F32, tag="mne",
                                               name="mne")
                nc.tensor.matmul(
                    out=state["mne"][:, ti:ti + 1],
                    lhsT=ohe_sb[:, t * P:(t + 1) * P],
                    rhs=mnegb_w[:, g:g + 1],
                    start=True, stop=True)
                e_t = tiny.tile([P, 1], F32, tag="e")
                nc.scalar.activation(
                    out=e_t[:], in_=state["mne"][:, ti:ti + 1],
                    func=mybir.ActivationFunctionType.Exp,
                    bias=s_wide[:, t:t + 1], scale=1.0)
                ohp = ohpp.tile([P, P], F32R, tag="ohp")
                nc.vector.tensor_scalar(
                    out=ohp[:],
                    in0=iota_sb[:],
                    scalar1=segloc_sb[:, t:t + 1],
                    scalar2=e_t[:],
                    op0=mybir.AluOpType.is_equal,
                    op1=mybir.AluOpType.mult)
                state[("ohp", ti)] = ohp

            def phase3_pool(g, ti, state):
                t = g * TPG + ti
                x_t = live.pop(t)
                ohp = state.pop(("ohp", ti))
                if ti == 0:
                    state["bagwin"] = bagwin_ps.tile(
                        [P, C], F32, tag="bagwin", name="bagwin")
                bagwin = state["bagwin"]
                for h in range(2):
                    nc.tensor.matmul(
                        out=bagwin[:, h * 512:(h + 1) * 512],
                        lhsT=ohp[:],
                        rhs=x_t[:, h * 512:(h + 1) * 512],
                        start=(ti == 0), stop=(ti == TPG - 1))
                if ti == TPG - 1:
                    # free bagwin ASAP: cast to fp16 right behind the last MM
                    bagn16 = bagnp.tile([P, C], F16, tag="bagn16")
                    nc.scalar.activation(
                        out=bagn16[:], in_=bagwin[:],
                        func=mybir.ActivationFunctionType.Copy)
                    state["bagn16"] = bagn16

            def phase3_tail_b(g, state):
                btp = bagt_ps.tile([P, C], F16, tag="bagtps", name="bagtps")
                bagn16 = state["bagn16"]
                for cch in range(NCH):
                    nc.tensor.transpose(
                        out=btp[:, cch * P:(cch + 1) * P],
                        in_=bagn16[:, cch * P:(cch + 1) * P],
                        identity=ident16[:])
                bagt = bagtp.tile([P, C], F16, tag="bagt")
                nc.scalar.copy(out=bagt[:], in_=btp[:])
                state["bagt"] = bagt

            def phase3_tail_c(g, state):
                mcls = conper_ps.tile([P, C], F32, tag="cp", name="mcls")
                bagt = state["bagt"]
                for cch in range(NCH):
                    nc.tensor.matmul(
                        out=mcls[:, 0:R],
                        lhsT=bagt[:, cch * P:(cch + 1) * P],
                        rhs=wt_sb[:, cch * R:(cch + 1) * R],
                        start=(cch == 0), stop=(cch == NCH - 1))
                nc.scalar.activation(
                    out=cls_acc[:, g * R:(g + 1) * R], in_=mcls[:, 0:R],
                    func=mybir.ActivationFunctionType.Copy,
                    scale=rw_w[:, g:g + 1])
                if (g + 1) % GPC == 0 or g == NG - 1:
                    c0 = (g // GPC) * GPC * R
                    c1 = (g + 1) * R
                    nc.sync.dma_start(out=outg[:, c0:c1],
                                      in_=cls_acc[:, c0:c1])

            # ---------------- pipeline ----------------
            states = {}
            tailq = []

            def pop_tail():
                if tailq:
                    fn, gg = tailq.pop(0)
                    fn(gg, states[gg])

            def slot(g, drain_g=None):
                """One group slot: 4 tiles of phase1 (if g < NG) interleaved
                with phase3 of group gg = (drain_g or g - LAG)."""
                gg = drain_g if drain_g is not None else g - LAG
                for ti in range(TPG):
                    if 0 <= gg < NG:
                        if ti == 0:
                            states[gg] = {}
                        phase3_head(gg, ti, states[gg])
                        if ti >= 1:
                            phase3_pool(gg, ti - 1, states[gg])
                    if g < NG:
                        phase1_tile(g * TPG + ti)
                    pop_tail()
                if 0 <= gg < NG:
                    phase3_pool(gg, TPG - 1, states[gg])
                if g < NG:
                    if g % 2 == 1:
                        s_flush(g - 1, g)
                    elif g == NG - 1:
                        s_flush(g, g)
                    for k in range(NG - 1):
                        if k in s_batch and (k + 1) in s_batch \
                                and not phase2_done[k]:
                            phase2(k)
                    if g == NG - 1 and NG - 1 in s_batch:
                        phase2(NG - 1)
                if 0 <= gg < NG:
                    tailq.append((phase3_tail_b, gg))
                    tailq.append((phase3_tail_c, gg))

            for g in range(NG):
                slot(g)
            for gg in range(NG - LAG, NG):
                slot(NG, drain_g=gg)
            while tailq:
                pop_tail()
            states.clear()
            live.clear()

    nc.compile()
    return nc


_CACHE = {}


def _get_program(meta):
    key = tuple(sorted(meta.items()))
    if key not in _CACHE:
        _CACHE[key] = _build(meta)
    return _CACHE[key]


def kernel(X, Constraints, W, b, X_Scope, X_Rel):
    X = np.asarray(X)
    b_np = np.asarray(b, dtype=np.float32)
    meta, in_maps, combine = _prep(X, Constraints, W, b, X_Scope, X_Rel)
    nc = _get_program(meta)
    res = run_bass_kernel_spmd(nc, in_maps, core_ids=list(range(NCORES)))
    B = X_Scope.shape[0]
    R = Constraints.shape[0]
    NG = meta["NG"]
    out = np.zeros((B, R), dtype=np.float32)
    for k in range(NCORES):
        og = res.results[k]["outg"]                          # [P, NG*R]
        cb = combine[k]
        b0, wlo, whi = cb["b0"], cb["wlo"], cb["whi"]
        for g in range(NG):
            if whi[g] < 0:
                continue
            nreal = int(whi[g] - wlo[g]) + 1
            lo = b0 + int(wlo[g])
            out[lo:lo + nreal] += og[:nreal, g * R:(g + 1) * R]
    return out + b_np[None, :]
